# revision 1
# baseline (speedup 1.0000x reference)
"""Trainium2 Bass kernel for per-channel argmax box masking (local mask).

Semantics (matches the reference nn.Module):
  For each channel map m = x[b, c] of shape 56x56 (flattened 3136):
    idx = argmax(m); mi = idx // 56; mj = idx % 56
    h1 = clip(mi-3, 0, 55); h2 = clip(mi+3, 0, 55)   (exclusive upper)
    w1 = clip(mj-3, 0, 55); w2 = clip(mj+3, 0, 55)
    S = 1 everywhere, 0 inside box [h1,h2) x [w1,w2)
    lam = 3136 / (3136 - box_area)
    out = T[b,c] > 0 ? m * S * lam : m

Strategy: pure data-parallel over 8 NeuronCores (2048 channels each),
channel -> SBUF partition, two 128-channel groups per iteration.

Per group:
  - hierarchical argmax: one full tensor_reduce(max) over [128,56,56]
    gives row maxima; a global reduce + max_index on the 56 row maxima
    gives the argmax ROW (mi) after only one full scan.
  - a 6-row window starting at rs=clip(mi-3,0,50) is gathered from x in
    DRAM by indirect DMA (the window always contains the argmax), and a
    max_index on those 336 elements recovers the argmax COLUMN (mj).
  - a tiny ALU chain derives the box, lam and scale factors.
  - ACT scales the whole tile in place by (marked ? lam : 1); the tile
    is stored from the ACT HWDGE queue (no cross-engine wait).
  - the window values are multiplied by the precise mask (zero inside
    the box, scale elsewhere) and scattered back over the stored tile
    by indirect DMA; outside the box the scattered bytes equal the
    stored bytes exactly, so only in-box bytes change. Each iteration
    owns a private output DRAM tensor, so its scatter only orders
    against its own stores and the pipeline keeps flowing.
"""

import numpy as np

import concourse.bass as bass
import concourse.bacc as bacc
import concourse.mybir as mybir
import concourse.tile as tile
from contextlib import ExitStack

F32 = mybir.dt.float32
I32 = mybir.dt.int32
U32 = mybir.dt.uint32

H = 56
HW = H * H          # 3136
WIN = 6 * H         # 336  (6-row window always contains the box rows)
N_CORES = 8
CH_PER_CORE = 2048  # 32*512 / 8
ALU = mybir.AluOpType
ACTF = mybir.ActivationFunctionType
NEG_INF = -3.4e38


def build_kernel(n_groups: int = 16):
    """Build the per-core Bass program. n_groups 128-channel groups."""
    assert n_groups % 2 == 0
    n_iters = n_groups // 2
    nch = n_groups * 128
    nc = bacc.Bacc("TRN2", target_bir_lowering=False, debug=False)

    x = nc.dram_tensor("x", [nch, HW], F32, kind="ExternalInput").ap()
    tm = nc.dram_tensor("tm", [128, n_groups], F32, kind="ExternalInput").ap()
    gbg = nc.dram_tensor("gbg", [128, n_groups], F32, kind="ExternalInput").ap()
    gbl = nc.dram_tensor("gbl", [128, n_groups], F32, kind="ExternalInput").ap()
    crow = nc.dram_tensor("crow", [128, 6], F32, kind="ExternalInput").ap()
    ccol = nc.dram_tensor("ccol", [128, H], F32, kind="ExternalInput").ap()
    outs = [
        nc.dram_tensor(f"out{i}", [256, HW], F32, kind="ExternalOutput").ap()
        for i in range(n_iters)
    ]

    # channel-major views: [p, group, elem] and row views for indirect DMA
    x_g = x.rearrange("(n p) f -> p n f", p=128)
    x_rows = x.rearrange("a (r c) -> (a r) c", c=H)      # [nch*56, 56]
    out_g = [o.rearrange("(n p) f -> p n f", p=128) for o in outs]
    out_rows = [o.rearrange("a (r c) -> (a r) c", c=H) for o in outs]

    with ExitStack() as ctx:
        tc = ctx.enter_context(tile.TileContext(nc))
        cpool = ctx.enter_context(tc.tile_pool(name="consts", bufs=1))
        xpool = ctx.enter_context(tc.tile_pool(name="xtiles", bufs=4))
        wpool = ctx.enter_context(tc.tile_pool(name="wins", bufs=4))
        mpool = ctx.enter_context(tc.tile_pool(name="masks", bufs=6))
        spool = ctx.enter_context(tc.tile_pool(name="scalars", bufs=6))

        # constants, loaded once (off the sync queue which feeds x loads)
        crow_t = cpool.tile([128, 6], F32)
        ccol_t = cpool.tile([128, H], F32)
        tm_t = cpool.tile([128, n_groups], F32)
        gbg_t = cpool.tile([128, n_groups], F32)
        gbl_t = cpool.tile([128, n_groups], F32)
        nc.scalar.dma_start(crow_t[:], crow)
        nc.scalar.dma_start(ccol_t[:], ccol)
        nc.scalar.dma_start(tm_t[:], tm)
        nc.scalar.dma_start(gbg_t[:], gbg)
        nc.scalar.dma_start(gbl_t[:], gbl)

        # prewarm the ACT tables (Copy + Identity) so real activations are fast
        warm = cpool.tile([128, 1], F32)
        nc.vector.memset(warm[:], 1.0)
        nc.scalar.activation(warm[:], warm[:], ACTF.Copy, bias=0.0, scale=1.0)
        nc.scalar.activation(warm[:], warm[:], ACTF.Identity, bias=warm[:],
                             scale=1.0)

        ts = nc.vector.tensor_scalar
        tt = nc.vector.tensor_tensor

        # scatter for iteration i is emitted during iteration i+1, after its
        # gathers: by then store_i has completed, so the scatter never holds
        # the in-order Pool sequencer (head-of-line) while waiting.
        pending_scatter = []

        def flush_scatter():
            # one index per partition per scatter: HW SWDGE pairs each
            # partition with a single index and a single contiguous run.
            while pending_scatter:
                it, gid, wo = pending_scatter.pop(0)
                for g in range(2):
                    nc.gpsimd.indirect_dma_start(
                        out=out_rows[it],
                        out_offset=bass.IndirectOffsetOnAxis(
                            ap=gid[:, g : g + 1], axis=0
                        ),
                        in_=wo[:, g * WIN : (g + 1) * WIN],
                        in_offset=None,
                    )

        for i in range(n_iters):
            j0 = 2 * i

            xt = xpool.tile([128, 2 * HW], F32)
            xt3 = xt[:].rearrange("p (g f) -> p g f", f=HW)
            nc.sync.dma_start(xt3, x_g[:, j0 : j0 + 2, :])

            xw = wpool.tile([128, 2 * WIN], F32, tag="xw")
            woutp = wpool.tile([128, 2 * WIN], F32, tag="woutp")
            gidxs = spool.tile([128, 2], I32, tag="gidxs")

            def sc(tag, w=2):
                return spool.tile([128, w], F32, tag=tag, name=tag)

            mib, h1b, rsb, mjb = sc("mib"), sc("h1b"), sc("rsb"), sc("mjb")
            m8s = []

            # ---- A: row argmax per group (DVE) + gather issue ----
            for g in range(2):
                j = j0 + g
                xg3 = xt[:, g * HW : (g + 1) * HW].rearrange(
                    "p (r c) -> p r c", c=H
                )
                red56 = mpool.tile([128, H], F32, tag="red56")
                m8 = mpool.tile([128, 8], F32, tag="m8")
                idxr = spool.tile([128, 8], U32, tag="idxr")
                nc.vector.tensor_reduce(red56[:], xg3, mybir.AxisListType.X,
                                        ALU.max)
                nc.vector.memset(m8[:], NEG_INF)
                nc.vector.tensor_reduce(m8[:, 0:1], red56[:],
                                        mybir.AxisListType.X, ALU.max)
                nc.vector.max_index(idxr[:], m8[:], red56[:])
                m8s.append(m8)

                mi = mib[:, g : g + 1]
                h1 = h1b[:, g : g + 1]
                rs = rsb[:, g : g + 1]
                nc.vector.tensor_copy(mi, idxr[:, 0:1])
                ts(h1, mi, -3.0, 0.0, ALU.add, ALU.max)
                ts(rs, h1, 50.0, None, ALU.min)
                gf = sc("gf", 1)
                tt(gf[:], rs, gbg_t[:, j : j + 1], ALU.add)
                gidxg = spool.tile([128, 1], I32, tag="gidxg")
                nc.vector.tensor_copy(gidxg[:], gf[:])
                tt(gf[:], rs, gbl_t[:, j : j + 1], ALU.add)
                nc.vector.tensor_copy(gidxs[:, g : g + 1], gf[:])

                # window gather starts as soon as rs is known
                nc.gpsimd.indirect_dma_start(
                    out=xw[:, g * WIN : (g + 1) * WIN],
                    out_offset=None,
                    in_=x_rows,
                    in_offset=bass.IndirectOffsetOnAxis(ap=gidxg[:], axis=0),
                )
                if g == 1:
                    flush_scatter()

            # ---- B: column argmax from the gathered windows (DVE) ----
            for g in range(2):
                idxw = spool.tile([128, 8], U32, tag="idxw")
                nc.vector.max_index(idxw[:], m8s[g][:],
                                    xw[:, g * WIN : (g + 1) * WIN])
                nc.vector.tensor_copy(mjb[:, g : g + 1], idxw[:, 0:1])

            # ---- C: batched box/scale params (DVE small ops) ----
            # mj = widx - 56*(mi - rs): no mod op needed, quotient is known
            dd = sc("dd")
            tt(dd[:], mib[:], rsb[:], ALU.subtract)
            nc.vector.scalar_tensor_tensor(
                mjb[:], dd[:], -56.0, mjb[:], ALU.mult, ALU.add)
            h2 = sc("h2")
            ts(h2[:], mib[:], 3.0, 55.0, ALU.add, ALU.min)
            aa = sc("aa")
            tt(aa[:], h1b[:], rsb[:], ALU.subtract)
            bb = sc("bb")
            tt(bb[:], h2[:], rsb[:], ALU.subtract)
            bh = sc("bh")
            tt(bh[:], h2[:], h1b[:], ALU.subtract)
            w1 = sc("w1")
            ts(w1[:], mjb[:], -3.0, 0.0, ALU.add, ALU.max)
            w2 = sc("w2")
            ts(w2[:], mjb[:], 3.0, 55.0, ALU.add, ALU.min)
            bw = sc("bw")
            tt(bw[:], w2[:], w1[:], ALU.subtract)
            area = sc("area")
            tt(area[:], bh[:], bw[:], ALU.mult)
            den = sc("den")
            ts(den[:], area[:], -1.0, float(HW), ALU.mult, ALU.add)
            mk = tm_t[:, j0 : j0 + 2]
            rcp = sc("rcp")
            nc.vector.reciprocal(rcp[:], den[:])
            lam1 = sc("lam1")
            ts(lam1[:], rcp[:], float(HW), -1.0, ALU.mult, ALU.add)  # lam-1
            vv = sc("vv")
            tt(vv[:], lam1[:], mk, ALU.mult)                  # marked*(lam-1)
            sceff = sc("sceff")
            ts(sceff[:], vv[:], 1.0, None, ALU.add)           # marked?lam:1
            bneg = sc("bneg")
            tt(bneg[:], vv[:], mk, ALU.add)                   # marked*lam
            ts(bneg[:], bneg[:], -1.0, None, ALU.mult)

            # ---- D: masks (DVE), window values (ACT), scale (ACT) ----
            for g in range(2):
                sceff_g = sceff[:, g : g + 1]
                rm = mpool.tile([128, 6], F32, tag="rm")
                cm = mpool.tile([128, H], F32, tag="cm")
                ts(rm[:], crow_t[:], aa[:, g : g + 1], None, ALU.is_ge)
                nc.vector.scalar_tensor_tensor(
                    rm[:], crow_t[:], bb[:, g : g + 1], rm[:],
                    ALU.is_lt, ALU.mult)
                ts(rm[:], rm[:], bneg[:, g : g + 1], None, ALU.mult)
                ts(cm[:], ccol_t[:], w1[:, g : g + 1], None, ALU.is_ge)
                nc.vector.scalar_tensor_tensor(
                    cm[:], ccol_t[:], w2[:, g : g + 1], cm[:],
                    ALU.is_lt, ALU.mult)
                mwin = mpool.tile([128, WIN], F32, tag="mwin")
                for r in range(6):
                    nc.scalar.activation(mwin[:, r * H : (r + 1) * H], cm[:],
                                         ACTF.Identity,
                                         bias=sceff_g, scale=rm[:, r : r + 1])
                xg = xt[:, g * HW : (g + 1) * HW]
                nc.scalar.activation(xg, xg, ACTF.Copy, bias=0.0,
                                     scale=sceff_g)
                nc.gpsimd.tensor_tensor(
                    woutp[:, g * WIN : (g + 1) * WIN],
                    xw[:, g * WIN : (g + 1) * WIN], mwin[:], ALU.mult)

            # ---- store both groups; window rewrite deferred one iteration ----
            nc.scalar.dma_start(out_g[i][:, 0:2, :], xt3)
            pending_scatter.append((i, gidxs, woutp))

        flush_scatter()

    nc.compile()
    return nc


def host_inputs(x_core: np.ndarray, marked_core: np.ndarray, n_groups: int):
    """Per-core input map. x_core [nch, 3136] f32, marked_core [nch] f32."""
    nch = n_groups * 128
    assert x_core.shape == (nch, HW)
    p = np.arange(128, dtype=np.float32)[:, None]
    j = np.arange(n_groups, dtype=np.float32)[None, :]
    gbg = j * (128 * H) + p * H    # global row of channel (j*128+p)
    gbl = (j % 2) * (128 * H) + p * H  # row within the iteration's out tensor
    crow = np.broadcast_to(np.arange(6, dtype=np.float32), (128, 6)).copy()
    ccol = np.broadcast_to(np.arange(H, dtype=np.float32), (128, H)).copy()
    tm = np.ascontiguousarray(marked_core.reshape(n_groups, 128).T)
    return {
        "x": np.ascontiguousarray(x_core, dtype=np.float32),
        "tm": tm.astype(np.float32),
        "gbg": gbg.astype(np.float32),
        "gbl": gbl.astype(np.float32),
        "crow": crow.astype(np.float32),
        "ccol": ccol.astype(np.float32),
    }


_CACHE = {}


def _get_nc(n_groups: int):
    if n_groups not in _CACHE:
        _CACHE[n_groups] = build_kernel(n_groups)
    return _CACHE[n_groups]


def kernel(x: np.ndarray, T: np.ndarray, _trace: bool = False):
    from concourse.bass_utils import run_bass_kernel_spmd

    B, C, Hh, Ww = x.shape
    assert (Hh, Ww) == (H, H) and B * C == N_CORES * CH_PER_CORE
    xf = np.ascontiguousarray(np.asarray(x, dtype=np.float32)).reshape(B * C, HW)
    marked = (np.asarray(T).reshape(-1) > 0).astype(np.float32)

    n_groups = CH_PER_CORE // 128
    n_iters = n_groups // 2
    nc = _get_nc(n_groups)
    in_maps = [
        host_inputs(
            xf[c * CH_PER_CORE : (c + 1) * CH_PER_CORE],
            marked[c * CH_PER_CORE : (c + 1) * CH_PER_CORE],
            n_groups,
        )
        for c in range(N_CORES)
    ]
    res = run_bass_kernel_spmd(nc, in_maps, list(range(N_CORES)), trace=_trace)
    out = np.concatenate(
        [res.results[c][f"out{i}"] for c in range(N_CORES) for i in range(n_iters)],
        axis=0,
    )
    out = out.reshape(B, C, Hh, Ww).astype(np.float32)
    if _trace:
        return out, res
    return out



# revision 2
# speedup vs baseline: 1.4364x; 1.4364x over previous
"""Trainium2 Bass kernel for per-channel argmax box masking (local mask).

Semantics (matches the reference nn.Module):
  For each channel map m = x[b, c] of shape 56x56 (flattened 3136):
    idx = argmax(m); mi = idx // 56; mj = idx % 56
    h1 = clip(mi-3, 0, 55); h2 = clip(mi+3, 0, 55)   (exclusive upper)
    w1 = clip(mj-3, 0, 55); w2 = clip(mj+3, 0, 55)
    S = 1 everywhere, 0 inside box [h1,h2) x [w1,w2)
    lam = 3136 / (3136 - box_area)
    out = T[b,c] > 0 ? m * S * lam : m

Sharding strategy: channels with T == 0 are a pure identity (out == x), so
the host routes them straight into the output and only ships the ~50%
marked channels to the device, balanced across the 8 cores (padded to a
multiple of 256 per core). The device kernel computes the masked+scaled
values for its channels and returns them as fp16 (well inside the 2e-2
relative-error budget); unmarked channels stay bit-exact f32 on host.

Per 128-channel group on device:
  - hierarchical argmax: one full tensor_reduce(max) over [128,56,56]
    gives row maxima; a global reduce + max_index on the 56 row maxima
    gives the argmax ROW (mi) after only one full scan.
  - a 6-row window starting at rs=clip(mi-3,0,50) is gathered from x in
    DRAM by indirect DMA (the window always contains the argmax), and a
    max_index on those 336 elements recovers the argmax COLUMN (mj).
  - a tiny ALU chain derives the box, lam and scale factors.
  - ACT writes the scaled tile (x * (marked ? lam : 1)) to an fp16 tile,
    stored from the ACT HWDGE queue (no cross-engine wait).
  - the window values are multiplied by the precise mask (zero inside
    the box, scale elsewhere) and scattered back over the stored tile
    by indirect DMA; each iteration owns a private output DRAM tensor,
    so its scatter only orders against its own stores and the pipeline
    keeps flowing.
"""

import numpy as np

import concourse.bass as bass
import concourse.bacc as bacc
import concourse.mybir as mybir
import concourse.tile as tile
from contextlib import ExitStack

F32 = mybir.dt.float32
F16 = mybir.dt.float16
I32 = mybir.dt.int32
U32 = mybir.dt.uint32

H = 56
HW = H * H          # 3136
WIN = 6 * H         # 336  (6-row window always contains the box rows)
N_CORES = 8
ALU = mybir.AluOpType
ACTF = mybir.ActivationFunctionType
NEG_INF = -3.4e38


def build_kernel(n_groups: int):
    """Build the per-core Bass program for n_groups 128-channel groups."""
    assert n_groups % 2 == 0
    n_iters = n_groups // 2
    nch = n_groups * 128
    nc = bacc.Bacc("TRN2", target_bir_lowering=False, debug=False)

    x = nc.dram_tensor("x", [nch, HW], F32, kind="ExternalInput").ap()
    tm = nc.dram_tensor("tm", [128, n_groups], F32, kind="ExternalInput").ap()
    gbg = nc.dram_tensor("gbg", [128, n_groups], F32, kind="ExternalInput").ap()
    gbl = nc.dram_tensor("gbl", [128, n_groups], F32, kind="ExternalInput").ap()
    crow = nc.dram_tensor("crow", [128, 6], F32, kind="ExternalInput").ap()
    ccol = nc.dram_tensor("ccol", [128, H], F32, kind="ExternalInput").ap()
    outs = [
        nc.dram_tensor(f"out{i}", [256, HW], F16, kind="ExternalOutput").ap()
        for i in range(n_iters)
    ]

    # channel-major views: [p, group, elem] and row views for indirect DMA
    x_g = x.rearrange("(n p) f -> p n f", p=128)
    x_rows = x.rearrange("a (r c) -> (a r) c", c=H)      # [nch*56, 56]
    out_g = [o.rearrange("(n p) f -> p n f", p=128) for o in outs]
    out_rows = [o.rearrange("a (r c) -> (a r) c", c=H) for o in outs]

    with ExitStack() as ctx:
        tc = ctx.enter_context(tile.TileContext(nc))
        cpool = ctx.enter_context(tc.tile_pool(name="consts", bufs=1))
        xpool = ctx.enter_context(tc.tile_pool(name="xtiles", bufs=4))
        opool = ctx.enter_context(tc.tile_pool(name="otiles", bufs=4))
        wpool = ctx.enter_context(tc.tile_pool(name="wins", bufs=4))
        mpool = ctx.enter_context(tc.tile_pool(name="masks", bufs=6))
        spool = ctx.enter_context(tc.tile_pool(name="scalars", bufs=6))

        # constants, loaded once (off the sync queue which feeds x loads)
        crow_t = cpool.tile([128, 6], F32)
        ccol_t = cpool.tile([128, H], F32)
        tm_t = cpool.tile([128, n_groups], F32)
        gbg_t = cpool.tile([128, n_groups], F32)
        gbl_t = cpool.tile([128, n_groups], F32)
        nc.scalar.dma_start(crow_t[:], crow)
        nc.scalar.dma_start(ccol_t[:], ccol)
        nc.scalar.dma_start(tm_t[:], tm)
        nc.scalar.dma_start(gbg_t[:], gbg)
        nc.scalar.dma_start(gbl_t[:], gbl)

        # prewarm the ACT tables (Copy + Identity) so real activations are fast
        warm = cpool.tile([128, 1], F32)
        nc.vector.memset(warm[:], 1.0)
        nc.scalar.activation(warm[:], warm[:], ACTF.Copy, bias=0.0, scale=1.0)
        nc.scalar.activation(warm[:], warm[:], ACTF.Identity, bias=warm[:],
                             scale=1.0)

        ts = nc.vector.tensor_scalar
        tt = nc.vector.tensor_tensor

        # scatter for iteration i is emitted during iteration i+1, after its
        # gathers: by then store_i has completed, so the scatter never holds
        # the in-order Pool sequencer (head-of-line) while waiting.
        pending_scatter = []

        def flush_scatter():
            # one index per partition per scatter: HW SWDGE pairs each
            # partition with a single index and a single contiguous run.
            while pending_scatter:
                it, gid, wo = pending_scatter.pop(0)
                for g in range(2):
                    nc.gpsimd.indirect_dma_start(
                        out=out_rows[it],
                        out_offset=bass.IndirectOffsetOnAxis(
                            ap=gid[:, g : g + 1], axis=0
                        ),
                        in_=wo[:, g * WIN : (g + 1) * WIN],
                        in_offset=None,
                    )

        for i in range(n_iters):
            j0 = 2 * i

            # separate tile + load per group: the group-0 reduce starts as
            # soon as its own 1.6MB load lands, not after both.
            xts = []
            for g in range(2):
                xt = xpool.tile([128, HW], F32, tag=f"xt{g}")
                nc.sync.dma_start(xt[:], x_g[:, j0 + g, :])
                xts.append(xt)

            xw = wpool.tile([128, 2 * WIN], F32, tag="xw")
            woutp = wpool.tile([128, 2 * WIN], F16, tag="woutp")
            gidxs = spool.tile([128, 2], I32, tag="gidxs")

            def sc(tag, w=2):
                return spool.tile([128, w], F32, tag=tag, name=tag)

            mib, h1b, rsb, mjb = sc("mib"), sc("h1b"), sc("rsb"), sc("mjb")
            m8s = []

            # ---- A: row argmax per group (DVE) + gather issue ----
            for g in range(2):
                j = j0 + g
                xg3 = xts[g][:].rearrange("p (r c) -> p r c", c=H)
                red56 = mpool.tile([128, H], F32, tag="red56")
                m8 = mpool.tile([128, 8], F32, tag="m8")
                idxr = spool.tile([128, 8], U32, tag="idxr")
                nc.vector.tensor_reduce(red56[:], xg3, mybir.AxisListType.X,
                                        ALU.max)
                nc.vector.memset(m8[:], NEG_INF)
                nc.vector.tensor_reduce(m8[:, 0:1], red56[:],
                                        mybir.AxisListType.X, ALU.max)
                nc.vector.max_index(idxr[:], m8[:], red56[:])
                m8s.append(m8)

                mi = mib[:, g : g + 1]
                h1 = h1b[:, g : g + 1]
                rs = rsb[:, g : g + 1]
                nc.vector.tensor_copy(mi, idxr[:, 0:1])
                ts(h1, mi, -3.0, 0.0, ALU.add, ALU.max)
                ts(rs, h1, 50.0, None, ALU.min)
                gf = sc("gf", 1)
                tt(gf[:], rs, gbg_t[:, j : j + 1], ALU.add)
                gidxg = spool.tile([128, 1], I32, tag="gidxg")
                nc.vector.tensor_copy(gidxg[:], gf[:])
                tt(gf[:], rs, gbl_t[:, j : j + 1], ALU.add)
                nc.vector.tensor_copy(gidxs[:, g : g + 1], gf[:])

                # window gather starts as soon as rs is known
                nc.gpsimd.indirect_dma_start(
                    out=xw[:, g * WIN : (g + 1) * WIN],
                    out_offset=None,
                    in_=x_rows,
                    in_offset=bass.IndirectOffsetOnAxis(ap=gidxg[:], axis=0),
                )
                if g == 1:
                    flush_scatter()

            # ---- B: column argmax from the gathered windows (DVE) ----
            for g in range(2):
                idxw = spool.tile([128, 8], U32, tag="idxw")
                nc.vector.max_index(idxw[:], m8s[g][:],
                                    xw[:, g * WIN : (g + 1) * WIN])
                nc.vector.tensor_copy(mjb[:, g : g + 1], idxw[:, 0:1])

            # ---- C: batched box/scale params (DVE small ops) ----
            # mj = widx - 56*(mi - rs): no mod op needed, quotient is known
            dd = sc("dd")
            tt(dd[:], mib[:], rsb[:], ALU.subtract)
            nc.vector.scalar_tensor_tensor(
                mjb[:], dd[:], -56.0, mjb[:], ALU.mult, ALU.add)
            h2 = sc("h2")
            ts(h2[:], mib[:], 3.0, 55.0, ALU.add, ALU.min)
            aa = sc("aa")
            tt(aa[:], h1b[:], rsb[:], ALU.subtract)
            bb = sc("bb")
            tt(bb[:], h2[:], rsb[:], ALU.subtract)
            bh = sc("bh")
            tt(bh[:], h2[:], h1b[:], ALU.subtract)
            w1 = sc("w1")
            ts(w1[:], mjb[:], -3.0, 0.0, ALU.add, ALU.max)
            w2 = sc("w2")
            ts(w2[:], mjb[:], 3.0, 55.0, ALU.add, ALU.min)
            bw = sc("bw")
            tt(bw[:], w2[:], w1[:], ALU.subtract)
            area = sc("area")
            tt(area[:], bh[:], bw[:], ALU.mult)
            den = sc("den")
            ts(den[:], area[:], -1.0, float(HW), ALU.mult, ALU.add)
            mk = tm_t[:, j0 : j0 + 2]
            rcp = sc("rcp")
            nc.vector.reciprocal(rcp[:], den[:])
            lam1 = sc("lam1")
            ts(lam1[:], rcp[:], float(HW), -1.0, ALU.mult, ALU.add)  # lam-1
            vv = sc("vv")
            tt(vv[:], lam1[:], mk, ALU.mult)                  # marked*(lam-1)
            sceff = sc("sceff")
            ts(sceff[:], vv[:], 1.0, None, ALU.add)           # marked?lam:1
            bneg = sc("bneg")
            tt(bneg[:], vv[:], mk, ALU.add)                   # marked*lam
            ts(bneg[:], bneg[:], -1.0, None, ALU.mult)

            # ---- D: masks (DVE), window values (Pool), scale (ACT) ----
            for g in range(2):
                sceff_g = sceff[:, g : g + 1]
                rm = mpool.tile([128, 6], F32, tag="rm")
                cm = mpool.tile([128, H], F32, tag="cm")
                ts(rm[:], crow_t[:], aa[:, g : g + 1], None, ALU.is_ge)
                nc.vector.scalar_tensor_tensor(
                    rm[:], crow_t[:], bb[:, g : g + 1], rm[:],
                    ALU.is_lt, ALU.mult)
                ts(rm[:], rm[:], bneg[:, g : g + 1], None, ALU.mult)
                ts(cm[:], ccol_t[:], w1[:, g : g + 1], None, ALU.is_ge)
                nc.vector.scalar_tensor_tensor(
                    cm[:], ccol_t[:], w2[:, g : g + 1], cm[:],
                    ALU.is_lt, ALU.mult)
                mwin = mpool.tile([128, WIN], F32, tag="mwin")
                for r in range(6):
                    nc.scalar.activation(mwin[:, r * H : (r + 1) * H], cm[:],
                                         ACTF.Identity,
                                         bias=sceff_g, scale=rm[:, r : r + 1])
                ot = opool.tile([128, HW], F16, tag=f"ot{g}")
                nc.scalar.activation(ot[:], xts[g][:], ACTF.Copy, bias=0.0,
                                     scale=sceff_g)
                nc.scalar.dma_start(out_g[i][:, g, :], ot[:])
                nc.gpsimd.tensor_tensor(
                    woutp[:, g * WIN : (g + 1) * WIN],
                    xw[:, g * WIN : (g + 1) * WIN], mwin[:], ALU.mult)

            # window rewrite deferred one iteration (see flush_scatter)
            pending_scatter.append((i, gidxs, woutp))

        flush_scatter()

    nc.compile()
    return nc


def host_inputs(x_core: np.ndarray, n_groups: int):
    """Per-core input map. x_core [nch, 3136] f32 (all channels marked)."""
    nch = n_groups * 128
    assert x_core.shape == (nch, HW)
    p = np.arange(128, dtype=np.float32)[:, None]
    j = np.arange(n_groups, dtype=np.float32)[None, :]
    gbg = j * (128 * H) + p * H    # global row of channel (j*128+p)
    gbl = (j % 2) * (128 * H) + p * H  # row within the iteration's out tensor
    crow = np.broadcast_to(np.arange(6, dtype=np.float32), (128, 6)).copy()
    ccol = np.broadcast_to(np.arange(H, dtype=np.float32), (128, H)).copy()
    tm = np.ones((128, n_groups), dtype=np.float32)
    return {
        "x": np.ascontiguousarray(x_core, dtype=np.float32),
        "tm": tm,
        "gbg": gbg.astype(np.float32),
        "gbl": gbl.astype(np.float32),
        "crow": crow,
        "ccol": ccol,
    }


_CACHE = {}


def _get_nc(n_groups: int):
    if n_groups not in _CACHE:
        _CACHE[n_groups] = build_kernel(n_groups)
    return _CACHE[n_groups]


def kernel(x: np.ndarray, T: np.ndarray, _trace: bool = False):
    from concourse.bass_utils import run_bass_kernel_spmd

    B, C, Hh, Ww = x.shape
    assert (Hh, Ww) == (H, H)
    nch_total = B * C
    xf = np.ascontiguousarray(np.asarray(x, dtype=np.float32)).reshape(
        nch_total, HW)
    mb = np.asarray(T).reshape(-1) > 0
    midx = np.flatnonzero(mb)
    n_m = int(midx.size)

    # output starts as a copy of x; only marked channels get overwritten
    out = xf.copy()

    if n_m > 0:
        # pad the marked set to N_CORES * (even number of 128-groups)
        n_groups = -(-n_m // (N_CORES * 128))   # ceil
        n_groups += n_groups % 2                # even (2 groups per iter)
        per_core = n_groups * 128
        total = per_core * N_CORES
        slot = np.concatenate(
            [midx, np.full(total - n_m, midx[0], dtype=midx.dtype)])

        nc = _get_nc(n_groups)
        in_maps = [
            host_inputs(xf[slot[c * per_core : (c + 1) * per_core]], n_groups)
            for c in range(N_CORES)
        ]
        res = run_bass_kernel_spmd(nc, in_maps, list(range(N_CORES)),
                                   trace=_trace)
        n_iters = n_groups // 2
        dev = np.concatenate(
            [res.results[c][f"out{i}"]
             for c in range(N_CORES) for i in range(n_iters)],
            axis=0,
        )
        out[midx] = dev[:n_m]   # fp16 -> f32 cast on assignment
    else:
        res = None

    out = out.reshape(B, C, Hh, Ww)
    if _trace:
        return out, res
    return out


# revision 8
# speedup vs baseline: 1.5265x; 1.0627x over previous
"""Trainium2 Bass kernel for per-channel argmax box masking (local mask).

Semantics (matches the reference nn.Module):
  For each channel map m = x[b, c] of shape 56x56 (flattened 3136):
    idx = argmax(m); mi = idx // 56; mj = idx % 56
    h1 = clip(mi-3, 0, 55); h2 = clip(mi+3, 0, 55)   (exclusive upper)
    w1 = clip(mj-3, 0, 55); w2 = clip(mj+3, 0, 55)
    S = 1 everywhere, 0 inside box [h1,h2) x [w1,w2)
    lam = 3136 / (3136 - box_area)
    out = T[b,c] > 0 ? m * S * lam : m

Sharding strategy: channels with T == 0 are a pure identity (out == x), so
the host routes them straight into the output and only ships the ~50%
marked channels to the device, balanced across the 8 cores (padded to a
multiple of 128 per core). The device kernel computes the masked+scaled
values for its channels and returns them as fp16 (well inside the 2e-2
relative-error budget); unmarked channels stay bit-exact f32 on host.

Per 128-channel group on device:
  - hierarchical argmax: one full tensor_reduce(max) over [128,56,56]
    gives row maxima; a global reduce + max_index on the 56 row maxima
    gives the argmax ROW (mi) after only one full scan.
  - a 6-row window starting at rs=clip(mi-3,0,50) is gathered from x in
    DRAM by indirect DMA (the window always contains the argmax), and a
    max_index on those 336 elements recovers the argmax COLUMN (mj).
  - a tiny ALU chain derives the box, lam and scale factors.
  - the window correction  woutp = (rm x cm + sceff) * xw  uses a
    stride-0 broadcast outer product on GpSimd plus one fused
    scalar_tensor_tensor on DVE.
  - ACT writes the scaled tile (x * lam) to an fp16 tile stored once per
    iteration (both groups interleaved per partition so each partition
    is one contiguous 12.5KB run); woutp and the window row starts are
    returned as small linear tensors and the HOST overlays the windows
    during unshard - no indirect scatter, no store-order tail.
"""

import numpy as np

import concourse.bass as bass
import concourse.bacc as bacc
import concourse.mybir as mybir
import concourse.tile as tile
from contextlib import ExitStack

F32 = mybir.dt.float32
F16 = mybir.dt.float16
I32 = mybir.dt.int32
U32 = mybir.dt.uint32

H = 56
HW = H * H          # 3136
WIN = 6 * H         # 336  (6-row window always contains the box rows)
N_CORES = 8
ALU = mybir.AluOpType
ACTF = mybir.ActivationFunctionType
NEG_INF = -3.4e38


def build_kernel(n_groups: int):
    """Build the per-core Bass program for n_groups 128-channel groups."""
    n_iters = (n_groups + 1) // 2
    widths = [min(2, n_groups - 2 * i) for i in range(n_iters)]
    nch = n_groups * 128
    nc = bacc.Bacc("TRN2", target_bir_lowering=False, debug=False)

    x = nc.dram_tensor("x", [nch, HW], F32, kind="ExternalInput").ap()
    tm = nc.dram_tensor("tm", [128, n_groups], F32, kind="ExternalInput").ap()
    gbg = nc.dram_tensor("gbg", [128, n_groups], F32, kind="ExternalInput").ap()
    crow = nc.dram_tensor("crow", [128, 6], F32, kind="ExternalInput").ap()
    ccol = nc.dram_tensor("ccol", [128, H], F32, kind="ExternalInput").ap()
    outs, wouts, rss = [], [], []
    for i in range(n_iters):
        w = widths[i]
        outs.append(nc.dram_tensor(f"out{i}", [w * 128, HW], F16,
                                   kind="ExternalOutput").ap())
        wouts.append(nc.dram_tensor(f"wout{i}", [128, w * WIN], F16,
                                    kind="ExternalOutput").ap())
        rss.append(nc.dram_tensor(f"rs{i}", [128, w], F32,
                                  kind="ExternalOutput").ap())

    # channel-major views: [p, group, elem] and a row view for the gather
    x_g = x.rearrange("(n p) f -> p n f", p=128)
    x_rows = x.rearrange("a (r c) -> (a r) c", c=H)      # [nch*56, 56]
    # out{i} row (p*w + g): partition p's groups are adjacent, so the
    # whole iteration stores as one 12.5KB contiguous run per partition.
    out_p = [o.rearrange("(p n) f -> p (n f)", p=128) for o in outs]

    with ExitStack() as ctx:
        tc = ctx.enter_context(tile.TileContext(nc))
        cpool = ctx.enter_context(tc.tile_pool(name="consts", bufs=1))
        xpool = ctx.enter_context(tc.tile_pool(name="xtiles", bufs=3))
        opool = ctx.enter_context(tc.tile_pool(name="otiles", bufs=3))
        wpool = ctx.enter_context(tc.tile_pool(name="wins", bufs=4))
        mpool = ctx.enter_context(
            tc.tile_pool(name="masks", bufs=2 * n_iters + 2))
        spool = ctx.enter_context(
            tc.tile_pool(name="scalars", bufs=2 * n_iters + 2))

        # constants, loaded once (off the sync queue which feeds x loads)
        crow_t = cpool.tile([128, 6], F32)
        ccol_t = cpool.tile([128, H], F32)
        tm_t = cpool.tile([128, n_groups], F32)
        gbg_t = cpool.tile([128, n_groups], F32)
        nc.scalar.dma_start(crow_t[:], crow)
        nc.scalar.dma_start(ccol_t[:], ccol)
        nc.scalar.dma_start(tm_t[:], tm)
        nc.scalar.dma_start(gbg_t[:], gbg)

        # prewarm the ACT table (Copy) so real activations are fast
        warm = cpool.tile([128, 1], F32)
        nc.vector.memset(warm[:], 1.0)
        nc.scalar.activation(warm[:], warm[:], ACTF.Copy, bias=0.0, scale=1.0)

        ts = nc.vector.tensor_scalar
        tt = nc.vector.tensor_tensor

        for i in range(n_iters):
            j0 = 2 * i
            w = widths[i]
            wt = "" if w == 2 else "T"   # tile-tag suffix for the odd tail

            # separate tile + load per group: the group-0 reduce starts as
            # soon as its own 1.6MB load lands, not after both.
            xts = []
            for g in range(w):
                xt = xpool.tile([128, HW], F32, tag=f"xt{g}")
                nc.sync.dma_start(xt[:], x_g[:, j0 + g, :])
                xts.append(xt)

            xw = wpool.tile([128, w * WIN], F32, tag="xw" + wt)
            woutp = wpool.tile([128, w * WIN], F16, tag="woutp" + wt)
            idxr = spool.tile([128, 8 * w], U32, tag="idxr" + wt)
            idxw = spool.tile([128, 8 * w], U32, tag="idxw" + wt)
            idxr3 = idxr[:].rearrange("p (g k) -> p g k", k=8)
            idxw3 = idxw[:].rearrange("p (g k) -> p g k", k=8)

            def sc(tag, width=w):
                return spool.tile([128, width], F32, tag=tag + wt, name=tag)

            mib, h1b, rsb, mjb = sc("mib"), sc("h1b"), sc("rsb"), sc("mjb")
            m8s = []

            # ---- A: row argmax per group (DVE) + gather issue ----
            for g in range(w):
                j = j0 + g
                xg3 = xts[g][:].rearrange("p (r c) -> p r c", c=H)
                red56 = mpool.tile([128, H], F32, tag="red56")
                m8 = mpool.tile([128, 8], F32, tag="m8")
                nc.vector.tensor_reduce(red56[:], xg3, mybir.AxisListType.X,
                                        ALU.max)
                nc.vector.memset(m8[:], NEG_INF)
                nc.vector.tensor_reduce(m8[:, 0:1], red56[:],
                                        mybir.AxisListType.X, ALU.max)
                nc.vector.max_index(idxr3[:, g, :], m8[:], red56[:])
                m8s.append(m8)

                mi = mib[:, g : g + 1]
                h1 = h1b[:, g : g + 1]
                rs = rsb[:, g : g + 1]
                nc.vector.tensor_copy(mi, idxr3[:, g, 0:1])
                ts(h1, mi, -3.0, 0.0, ALU.add, ALU.max)
                ts(rs, h1, 50.0, None, ALU.min)
                gidxg = spool.tile([128, 1], I32, tag="gidxg")
                tt(gidxg[:], rs, gbg_t[:, j : j + 1], ALU.add)

                # window gather starts as soon as rs is known
                nc.gpsimd.indirect_dma_start(
                    out=xw[:, g * WIN : (g + 1) * WIN],
                    out_offset=None,
                    in_=x_rows,
                    in_offset=bass.IndirectOffsetOnAxis(ap=gidxg[:], axis=0),
                )

            # the host needs the window row starts to overlay woutp
            nc.scalar.dma_start(rss[i], rsb[:])

            # ---- B/C interleaved: mjb-independent params run between the
            # two column argmaxes so the second gather's latency is hidden
            nc.vector.max_index(idxw3[:, 0, :], m8s[0][:], xw[:, 0:WIN])
            dd = sc("dd")
            tt(dd[:], mib[:], rsb[:], ALU.subtract)
            h2 = sc("h2")
            ts(h2[:], mib[:], 3.0, 55.0, ALU.add, ALU.min)
            aa = sc("aa")
            tt(aa[:], h1b[:], rsb[:], ALU.subtract)
            bb = sc("bb")
            tt(bb[:], h2[:], rsb[:], ALU.subtract)
            bh = sc("bh")
            tt(bh[:], h2[:], h1b[:], ALU.subtract)
            for g in range(1, w):
                nc.vector.max_index(idxw3[:, g, :], m8s[g][:],
                                    xw[:, g * WIN : (g + 1) * WIN])
            nc.vector.tensor_copy(mjb[:].unsqueeze(2), idxw3[:, :, 0:1])

            # mj = widx - 56*(mi - rs): no mod op needed, quotient is known
            nc.vector.scalar_tensor_tensor(
                mjb[:], dd[:], -56.0, mjb[:], ALU.mult, ALU.add)
            w1 = sc("w1")
            ts(w1[:], mjb[:], -3.0, 0.0, ALU.add, ALU.max)
            w2 = sc("w2")
            ts(w2[:], mjb[:], 3.0, 55.0, ALU.add, ALU.min)
            bw = sc("bw")
            tt(bw[:], w2[:], w1[:], ALU.subtract)
            area = sc("area")
            tt(area[:], bh[:], bw[:], ALU.mult)
            den = sc("den")
            ts(den[:], area[:], -1.0, float(HW), ALU.mult, ALU.add)
            mk = tm_t[:, j0 : j0 + w]
            rcp = sc("rcp")
            nc.vector.reciprocal(rcp[:], den[:])
            lam1 = sc("lam1")
            ts(lam1[:], rcp[:], float(HW), -1.0, ALU.mult, ALU.add)  # lam-1
            vv = sc("vv")
            tt(vv[:], lam1[:], mk, ALU.mult)                  # marked*(lam-1)
            sceff = sc("sceff")
            ts(sceff[:], vv[:], 1.0, None, ALU.add)           # marked?lam:1
            bneg = sc("bneg")
            tt(bneg[:], vv[:], mk, ALU.add)                   # marked*lam
            ts(bneg[:], bneg[:], -1.0, None, ALU.mult)

            # ---- D: masks (DVE), window correction (Pool+DVE), scale (ACT)
            ot = opool.tile([128, w * HW], F16, tag="ot" + wt)
            for g in range(w):
                sceff_g = sceff[:, g : g + 1]
                rm = mpool.tile([128, 6], F32, tag="rm")
                cm = mpool.tile([128, H], F32, tag="cm")
                ts(rm[:], crow_t[:], aa[:, g : g + 1], None, ALU.is_ge)
                nc.vector.scalar_tensor_tensor(
                    rm[:], crow_t[:], bb[:, g : g + 1], rm[:],
                    ALU.is_lt, ALU.mult)
                ts(rm[:], rm[:], bneg[:, g : g + 1], None, ALU.mult)
                ts(cm[:], ccol_t[:], w1[:, g : g + 1], None, ALU.is_ge)
                nc.vector.scalar_tensor_tensor(
                    cm[:], ccol_t[:], w2[:, g : g + 1], cm[:],
                    ALU.is_lt, ALU.mult)
                # mtmp[r, c] = rm[r] * cm[c]  (stride-0 broadcast outer)
                mtmp = mpool.tile([128, WIN], F32, tag="mtmp")
                mtmp3 = mtmp[:].rearrange("p (r c) -> p r c", c=H)
                nc.gpsimd.tensor_tensor(
                    mtmp3,
                    rm[:].unsqueeze(2).to_broadcast((128, 6, H)),
                    cm[:].unsqueeze(1).to_broadcast((128, 6, H)),
                    ALU.mult)
                # woutp = (mtmp + sceff) * xw fused on DVE (gpsimd
                # tensor_scalar with an AP scalar measures ~6us/op on HW)
                nc.vector.scalar_tensor_tensor(
                    woutp[:, g * WIN : (g + 1) * WIN],
                    mtmp[:], sceff_g, xw[:, g * WIN : (g + 1) * WIN],
                    ALU.add, ALU.mult)
                nc.scalar.activation(ot[:, g * HW : (g + 1) * HW], xts[g][:],
                                     ACTF.Copy, bias=0.0, scale=sceff_g)

            # one big store per iteration; windows go back as linear tensors
            nc.scalar.dma_start(out_p[i], ot[:])
            nc.scalar.dma_start(wouts[i], woutp[:])

    nc.compile()
    return nc


def host_inputs(x_core: np.ndarray, n_groups: int):
    """Per-core input map. x_core [nch, 3136] f32 (all channels marked)."""
    nch = n_groups * 128
    assert x_core.shape == (nch, HW)
    p = np.arange(128, dtype=np.float32)[:, None]
    j = np.arange(n_groups, dtype=np.float32)[None, :]
    gbg = j * (128 * H) + p * H    # global row of channel (j*128+p)
    crow = np.broadcast_to(np.arange(6, dtype=np.float32), (128, 6)).copy()
    ccol = np.broadcast_to(np.arange(H, dtype=np.float32), (128, H)).copy()
    tm = np.ones((128, n_groups), dtype=np.float32)
    return {
        "x": np.ascontiguousarray(x_core, dtype=np.float32),
        "tm": tm,
        "gbg": gbg.astype(np.float32),
        "crow": crow,
        "ccol": ccol,
    }


_CACHE = {}


def _get_nc(n_groups: int):
    if n_groups not in _CACHE:
        _CACHE[n_groups] = build_kernel(n_groups)
    return _CACHE[n_groups]


def kernel(x: np.ndarray, T: np.ndarray, _trace: bool = False):
    from concourse.bass_utils import run_bass_kernel_spmd

    B, C, Hh, Ww = x.shape
    assert (Hh, Ww) == (H, H)
    nch_total = B * C
    xf = np.ascontiguousarray(np.asarray(x, dtype=np.float32)).reshape(
        nch_total, HW)
    mb = np.asarray(T).reshape(-1) > 0
    midx = np.flatnonzero(mb)
    n_m = int(midx.size)

    # output starts as a copy of x; only marked channels get overwritten
    out = xf.copy()

    if n_m > 0:
        # pad the marked set to N_CORES * n_groups * 128 slots
        n_groups = -(-n_m // (N_CORES * 128))   # ceil
        per_core = n_groups * 128
        total = per_core * N_CORES
        slot = np.concatenate(
            [midx, np.full(total - n_m, midx[0], dtype=midx.dtype)])

        nc = _get_nc(n_groups)
        in_maps = [
            host_inputs(xf[slot[c * per_core : (c + 1) * per_core]], n_groups)
            for c in range(N_CORES)
        ]
        res = run_bass_kernel_spmd(nc, in_maps, list(range(N_CORES)),
                                   trace=_trace)

        n_iters = (n_groups + 1) // 2
        widths = [min(2, n_groups - 2 * i) for i in range(n_iters)]
        dev = np.empty((total, HW), dtype=np.float16)
        rs_all = np.empty(total, dtype=np.int64)
        wout_all = np.empty((total, WIN), dtype=np.float16)
        p = np.arange(128)
        for c in range(N_CORES):
            r = res.results[c]
            base = c * per_core
            for i, w in enumerate(widths):
                # out{i} row p*w+g  <->  slot (2i+g)*128 + p
                o = r[f"out{i}"]                      # [w*128, HW]
                wo = r[f"wout{i}"]                    # [128, w*WIN]
                rsv = r[f"rs{i}"]                     # [128, w]
                for g in range(w):
                    sl = base + (2 * i + g) * 128 + p
                    dev[sl] = o[p * w + g]
                    wout_all[sl] = wo[:, g * WIN : (g + 1) * WIN]
                    rs_all[sl] = rsv[:, g].astype(np.int64)
        # overlay the corrected 6-row windows at their per-channel rows
        col = rs_all[:, None] * H + np.arange(WIN)[None, :]
        np.put_along_axis(dev, col, wout_all, axis=1)
        out[midx] = dev[:n_m]   # fp16 -> f32 cast on assignment
    else:
        res = None

    out = out.reshape(B, C, Hh, Ww)
    if _trace:
        return out, res
    return out


# revision 15
# speedup vs baseline: 1.7439x; 1.1424x over previous
"""Trainium2 Bass kernel for per-channel argmax box masking (local mask).

Semantics (matches the reference nn.Module):
  For each channel map m = x[b, c] of shape 56x56 (flattened 3136):
    idx = argmax(m); mi = idx // 56; mj = idx % 56
    h1 = clip(mi-3, 0, 55); h2 = clip(mi+3, 0, 55)   (exclusive upper)
    w1 = clip(mj-3, 0, 55); w2 = clip(mj+3, 0, 55)
    S = 1 everywhere, 0 inside box [h1,h2) x [w1,w2)
    lam = 3136 / (3136 - box_area)
    out = T[b,c] > 0 ? m * S * lam : m

Sharding strategy: channels with T == 0 are a pure identity (out == x), so
the host routes them straight into the output and only ships the ~50%
marked channels to the device, balanced across the 8 cores (padded to a
multiple of 128 per core). The device kernel computes the masked+scaled
values for its channels and returns them as fp16 (well inside the 2e-2
relative-error budget); unmarked channels stay bit-exact f32 on host.

Per 128-channel group on device:
  - hierarchical argmax: one full tensor_reduce(max) over [128,56,56]
    gives row maxima; a global reduce + max_index on the 56 row maxima
    gives the argmax ROW (mi) after only one full scan.
  - a 6-row window starting at rs=clip(mi-3,0,50) is gathered from x in
    DRAM by indirect DMA (the window always contains the argmax), and a
    max_index on those 336 elements recovers the argmax COLUMN (mj).
  - a tiny ALU chain derives the box, lam and scale factors.
  - the window correction  woutp = (rm x cm + sceff) * xw  uses a
    stride-0 broadcast outer product on GpSimd plus one fused
    scalar_tensor_tensor on DVE.
  - ACT writes the scaled tile (x * lam) to an fp16 tile stored once per
    iteration (both groups interleaved per partition so each partition
    is one contiguous 12.5KB run); woutp and the window row starts are
    returned as small linear tensors and the HOST overlays the windows
    during unshard - no indirect scatter, no store-order tail.
"""

import numpy as np

import concourse.bass as bass
import concourse.bacc as bacc
import concourse.mybir as mybir
import concourse.tile as tile
from contextlib import ExitStack

F32 = mybir.dt.float32
F16 = mybir.dt.float16
I32 = mybir.dt.int32
U32 = mybir.dt.uint32

H = 56
HW = H * H          # 3136
WIN = 6 * H         # 336  (6-row window always contains the box rows)
N_CORES = 8
ALU = mybir.AluOpType
ACTF = mybir.ActivationFunctionType
NEG_INF = -3.4e38


def build_kernel(n_groups: int):
    """Build the per-core Bass program for n_groups 128-channel groups."""
    n_iters = (n_groups + 1) // 2
    widths = [min(2, n_groups - 2 * i) for i in range(n_iters)]
    nch = n_groups * 128
    nc = bacc.Bacc("TRN2", target_bir_lowering=False, debug=False)

    x = nc.dram_tensor("x", [nch, HW], F32, kind="ExternalInput").ap()
    outs, wouts, rss = [], [], []
    for i in range(n_iters):
        w = widths[i]
        outs.append(nc.dram_tensor(f"out{i}", [w * 128, HW], F16,
                                   kind="ExternalOutput").ap())
        wouts.append(nc.dram_tensor(f"wout{i}", [128, w * WIN], F16,
                                    kind="ExternalOutput").ap())
        rss.append(nc.dram_tensor(f"rs{i}", [128, w], F32,
                                  kind="ExternalOutput").ap())

    # channel-major views: [p, group, elem] and a row view for the gather
    x_g = x.rearrange("(n p) f -> p n f", p=128)
    x_rows = x.rearrange("a (r c) -> (a r) c", c=H)      # [nch*56, 56]
    # out{i} row (p*w + g): partition p's groups are adjacent, so the
    # whole iteration stores as one 12.5KB contiguous run per partition.
    out_p = [o.rearrange("(p n) f -> p (n f)", p=128) for o in outs]

    with ExitStack() as ctx:
        tc = ctx.enter_context(tile.TileContext(nc))
        cpool = ctx.enter_context(tc.tile_pool(name="consts", bufs=1))
        xpool = ctx.enter_context(tc.tile_pool(name="xtiles", bufs=3))
        opool = ctx.enter_context(tc.tile_pool(name="otiles", bufs=3))
        wpool = ctx.enter_context(tc.tile_pool(name="wins", bufs=4))
        mpool = ctx.enter_context(
            tc.tile_pool(name="masks", bufs=2 * n_iters + 2))
        spool = ctx.enter_context(
            tc.tile_pool(name="scalars", bufs=2 * n_iters + 2))

        # constants generated on device: a DMA-loaded constant's completion
        # semaphore can get batched behind x-load completions on a shared
        # lane, stalling its first reader ~13us (seen on HW traces).
        crow_t = cpool.tile([128, 6], F32)
        ccol_t = cpool.tile([128, H], F32)
        pio56 = cpool.tile([128, 1], F32)
        crow_i = cpool.tile([128, 6], I32)
        ccol_i = cpool.tile([128, H], I32)
        pio_i = cpool.tile([128, 1], I32)
        nc.gpsimd.iota(crow_i[:], [[1, 6]], base=0, channel_multiplier=0)
        nc.gpsimd.iota(ccol_i[:], [[1, H]], base=0, channel_multiplier=0)
        nc.gpsimd.iota(pio_i[:], [[0, 1]], base=0, channel_multiplier=H)
        nc.gpsimd.tensor_copy(crow_t[:], crow_i[:])
        nc.gpsimd.tensor_copy(ccol_t[:], ccol_i[:])
        nc.gpsimd.tensor_copy(pio56[:], pio_i[:])

        # prewarm the ACT table (Copy) so real activations are fast
        warm = cpool.tile([128, 1], F32)
        nc.vector.memset(warm[:], 1.0)
        nc.scalar.activation(warm[:], warm[:], ACTF.Copy, bias=0.0, scale=1.0)

        ts = nc.vector.tensor_scalar
        tt = nc.vector.tensor_tensor

        for i in range(n_iters):
            j0 = 2 * i
            w = widths[i]
            wt = "" if w == 2 else "T"   # tile-tag suffix for the odd tail

            # separate tile + load per group: the group-0 reduce starts as
            # soon as its own 1.6MB load lands, not after both.
            xts = []
            for g in range(w):
                xt = xpool.tile([128, HW], F32, tag=f"xt{g}")
                nc.sync.dma_start(xt[:], x_g[:, j0 + g, :])
                xts.append(xt)

            xw = wpool.tile([128, w * WIN], F32, tag="xw" + wt)
            woutp = wpool.tile([128, w * WIN], F16, tag="woutp" + wt)
            idxr = spool.tile([128, 8 * w], U32, tag="idxr" + wt)
            idxw = spool.tile([128, 8 * w], U32, tag="idxw" + wt)
            idxr3 = idxr[:].rearrange("p (g k) -> p g k", k=8)
            idxw3 = idxw[:].rearrange("p (g k) -> p g k", k=8)

            def sc(tag, width=w):
                return spool.tile([128, width], F32, tag=tag + wt, name=tag)

            mib, h1b, rsb, mjb = sc("mib"), sc("h1b"), sc("rsb"), sc("mjb")
            m8s = []

            # ---- A: row argmax per group (DVE) + gather issue ----
            for g in range(w):
                j = j0 + g
                xg3 = xts[g][:].rearrange("p (r c) -> p r c", c=H)
                red56 = mpool.tile([128, H], F32, tag="red56")
                m8 = mpool.tile([128, 8], F32, tag="m8")
                nc.vector.tensor_reduce(red56[:], xg3, mybir.AxisListType.X,
                                        ALU.max)
                nc.vector.memset(m8[:], NEG_INF)
                nc.vector.tensor_reduce(m8[:, 0:1], red56[:],
                                        mybir.AxisListType.X, ALU.max)
                nc.vector.max_index(idxr3[:, g, :], m8[:], red56[:])
                m8s.append(m8)

                mi = mib[:, g : g + 1]
                h1 = h1b[:, g : g + 1]
                rs = rsb[:, g : g + 1]
                nc.vector.tensor_copy(mi, idxr3[:, g, 0:1])
                ts(h1, mi, -3.0, 0.0, ALU.add, ALU.max)
                ts(rs, h1, 50.0, None, ALU.min)
                gidxg = spool.tile([128, 1], I32, tag="gidxg")
                # global gather row = rs + j*128*H/... + p*H, no DRAM consts
                nc.vector.scalar_tensor_tensor(
                    gidxg[:], rs, float(j * 128 * H), pio56[:],
                    ALU.add, ALU.add)

                # window gather starts as soon as rs is known
                nc.gpsimd.indirect_dma_start(
                    out=xw[:, g * WIN : (g + 1) * WIN],
                    out_offset=None,
                    in_=x_rows,
                    in_offset=bass.IndirectOffsetOnAxis(ap=gidxg[:], axis=0),
                )

            # the host needs the window row starts to overlay woutp; store
            # from the Pool queue so the ACT queue head never waits on rsb
            nc.gpsimd.dma_start(rss[i], rsb[:])

            # ---- B/C interleaved: mjb-independent params run between the
            # two column argmaxes so the second gather's latency is hidden
            nc.vector.max_index(idxw3[:, 0, :], m8s[0][:], xw[:, 0:WIN])
            dd = sc("dd")
            tt(dd[:], mib[:], rsb[:], ALU.subtract)
            h2 = sc("h2")
            ts(h2[:], mib[:], 3.0, 55.0, ALU.add, ALU.min)
            aa = sc("aa")
            tt(aa[:], h1b[:], rsb[:], ALU.subtract)
            bb = sc("bb")
            tt(bb[:], h2[:], rsb[:], ALU.subtract)
            bh = sc("bh")
            tt(bh[:], h2[:], h1b[:], ALU.subtract)
            for g in range(1, w):
                nc.vector.max_index(idxw3[:, g, :], m8s[g][:],
                                    xw[:, g * WIN : (g + 1) * WIN])
            nc.vector.tensor_copy(mjb[:].unsqueeze(2), idxw3[:, :, 0:1])

            # mj = widx - 56*(mi - rs): no mod op needed, quotient is known
            nc.vector.scalar_tensor_tensor(
                mjb[:], dd[:], -56.0, mjb[:], ALU.mult, ALU.add)
            w1 = sc("w1")
            ts(w1[:], mjb[:], -3.0, 0.0, ALU.add, ALU.max)
            w2 = sc("w2")
            ts(w2[:], mjb[:], 3.0, 55.0, ALU.add, ALU.min)
            bw = sc("bw")
            tt(bw[:], w2[:], w1[:], ALU.subtract)
            area = sc("area")
            tt(area[:], bh[:], bw[:], ALU.mult)
            den = sc("den")
            ts(den[:], area[:], -1.0, float(HW), ALU.mult, ALU.add)
            rcp = sc("rcp")
            nc.vector.reciprocal(rcp[:], den[:])
            # every device channel is marked by construction: sceff = lam
            sceff = sc("sceff")
            ts(sceff[:], rcp[:], float(HW), None, ALU.mult)
            bneg = sc("bneg")
            ts(bneg[:], sceff[:], -1.0, None, ALU.mult)

            # ---- D: masks (DVE), window correction (Pool+DVE), scale (ACT)
            ot = opool.tile([128, w * HW], F16, tag="ot" + wt)
            for g in range(w):
                sceff_g = sceff[:, g : g + 1]
                rm = mpool.tile([128, 6], F32, tag="rm")
                cm = mpool.tile([128, H], F32, tag="cm")
                ts(rm[:], crow_t[:], aa[:, g : g + 1], None, ALU.is_ge)
                nc.vector.scalar_tensor_tensor(
                    rm[:], crow_t[:], bb[:, g : g + 1], rm[:],
                    ALU.is_lt, ALU.mult)
                ts(rm[:], rm[:], bneg[:, g : g + 1], None, ALU.mult)
                ts(cm[:], ccol_t[:], w1[:, g : g + 1], None, ALU.is_ge)
                nc.vector.scalar_tensor_tensor(
                    cm[:], ccol_t[:], w2[:, g : g + 1], cm[:],
                    ALU.is_lt, ALU.mult)
                # mtmp[r, c] = rm[r] * cm[c]  (stride-0 broadcast outer)
                mtmp = mpool.tile([128, WIN], F32, tag="mtmp")
                mtmp3 = mtmp[:].rearrange("p (r c) -> p r c", c=H)
                nc.gpsimd.tensor_tensor(
                    mtmp3,
                    rm[:].unsqueeze(2).to_broadcast((128, 6, H)),
                    cm[:].unsqueeze(1).to_broadcast((128, 6, H)),
                    ALU.mult)
                # woutp = (mtmp + sceff) * xw fused on DVE (gpsimd
                # tensor_scalar with an AP scalar measures ~6us/op on HW)
                nc.vector.scalar_tensor_tensor(
                    woutp[:, g * WIN : (g + 1) * WIN],
                    mtmp[:], sceff_g, xw[:, g * WIN : (g + 1) * WIN],
                    ALU.add, ALU.mult)
                nc.scalar.activation(ot[:, g * HW : (g + 1) * HW], xts[g][:],
                                     ACTF.Copy, bias=0.0, scale=sceff_g)

            # one big store per iteration; windows go back as linear tensors
            nc.scalar.dma_start(out_p[i], ot[:])
            nc.scalar.dma_start(wouts[i], woutp[:])

    nc.compile()
    return nc


def host_inputs(x_core: np.ndarray, n_groups: int):
    """Per-core input map. x_core [nch, 3136] f32 (all channels marked)."""
    nch = n_groups * 128
    assert x_core.shape == (nch, HW)
    return {"x": np.ascontiguousarray(x_core, dtype=np.float32)}


_CACHE = {}


def _get_nc(n_groups: int):
    if n_groups not in _CACHE:
        _CACHE[n_groups] = build_kernel(n_groups)
    return _CACHE[n_groups]


def kernel(x: np.ndarray, T: np.ndarray, _trace: bool = False):
    from concourse.bass_utils import run_bass_kernel_spmd

    B, C, Hh, Ww = x.shape
    assert (Hh, Ww) == (H, H)
    nch_total = B * C
    xf = np.ascontiguousarray(np.asarray(x, dtype=np.float32)).reshape(
        nch_total, HW)
    mb = np.asarray(T).reshape(-1) > 0
    midx = np.flatnonzero(mb)
    n_m = int(midx.size)

    # output starts as a copy of x; only marked channels get overwritten
    out = xf.copy()

    if n_m > 0:
        # pad the marked set to N_CORES * n_groups * 128 slots
        n_groups = -(-n_m // (N_CORES * 128))   # ceil
        per_core = n_groups * 128
        total = per_core * N_CORES
        slot = np.concatenate(
            [midx, np.full(total - n_m, midx[0], dtype=midx.dtype)])

        nc = _get_nc(n_groups)
        in_maps = [
            host_inputs(xf[slot[c * per_core : (c + 1) * per_core]], n_groups)
            for c in range(N_CORES)
        ]
        res = run_bass_kernel_spmd(nc, in_maps, list(range(N_CORES)),
                                   trace=_trace)

        n_iters = (n_groups + 1) // 2
        widths = [min(2, n_groups - 2 * i) for i in range(n_iters)]
        dev = np.empty((total, HW), dtype=np.float16)
        rs_all = np.empty(total, dtype=np.int64)
        wout_all = np.empty((total, WIN), dtype=np.float16)
        p = np.arange(128)
        for c in range(N_CORES):
            r = res.results[c]
            base = c * per_core
            for i, w in enumerate(widths):
                # out{i} row p*w+g  <->  slot (2i+g)*128 + p
                o = r[f"out{i}"]                      # [w*128, HW]
                wo = r[f"wout{i}"]                    # [128, w*WIN]
                rsv = r[f"rs{i}"]                     # [128, w]
                for g in range(w):
                    sl = base + (2 * i + g) * 128 + p
                    dev[sl] = o[p * w + g]
                    wout_all[sl] = wo[:, g * WIN : (g + 1) * WIN]
                    rs_all[sl] = rsv[:, g].astype(np.int64)
        # overlay the corrected 6-row windows at their per-channel rows
        col = rs_all[:, None] * H + np.arange(WIN)[None, :]
        np.put_along_axis(dev, col, wout_all, axis=1)
        out[midx] = dev[:n_m]   # fp16 -> f32 cast on assignment
    else:
        res = None

    out = out.reshape(B, C, Hh, Ww)
    if _trace:
        return out, res
    return out
